# revision 1
# baseline (speedup 1.0000x reference)
"""MoE-routing attention kernel for 8 Trainium2 NeuronCores.

Expert parallelism (1 expert per core), full inputs in, full output out,
with token gathering: each core gathers only the tokens routed to its
expert (top-2 of 8; <= ~300 of 1024 per batch; capacity 384) and runs
projections/attention on the gathered set.

Per core, for its expert e:
  gate (fp32 PE): logits = x @ wg, softmax, top-2 -> mask[:,e], cw[:,e].
     The host permutes wg columns per core so column 0 is this core's
     expert (softmax/top-2 are permutation-equivariant).
  gather: exclusive prefix of the mask (lower-triangular matmul over
     partitions) -> slot positions; scatter token ids to an index list;
     indirect-DMA gather of x rows (pad slots stay zero via bounds
     checks). Zero pad rows make padded q,k equal the pure-bias rows of
     the reference's dense dispatch.
  q/k proj (fp32r PE) on [384] gathered slots; scores S[s,t] on the
     [384,384] gathered block; exp(S/D).
  weighting trick: the (T,T)-joint softmax terms for the T-C unassigned
     tokens are identical (bias-only rows/cols), so one zero pad slot
     weighted by (T-C) represents all of them (omega weights).
  v collapse: sum_d of the final output commutes through the output
     projection: sum_d out_e[t] = sum_s P[t,s]*vw[s] + sum(bo), with
     vw = x_t . (wv @ wo_sum) + bv.wo_sum computed by a DVE reduction.
  combine: scatter out_sum back to token space, multiply by cw.

Host: sums the 8 per-core [B,T] contribution vectors and applies the
final log_softmax (a 16KB reduction -- the SPMD combine/unshard step).

Capacity note: CAP=384 per (expert, batch) = mean 256 + 9.2 sigma for
top-2-of-8 routing; tokens beyond capacity would be dropped.
"""

import os
import sys

import numpy as np

for _p in ("/opt/trn_rl_repo", "/root/.axon_site/_ro/trn_rl_repo"):
    if _p not in sys.path:
        sys.path.append(_p)

import concourse.bass as bass  # noqa: E402
import concourse.bass_isa as bass_isa  # noqa: E402
import concourse.mybir as mybir  # noqa: E402
import concourse.tile as tile  # noqa: E402
from concourse import bacc  # noqa: E402
from concourse import bass_utils  # noqa: E402
from concourse.bass import ts  # noqa: E402
from concourse.masks import make_identity  # noqa: E402

P = 128
B, T, D, E = 4, 1024, 1024, 8
DH = D
N = B * T
DC = D // P  # 8 contraction chunks
FT = DH // P  # 8 f tiles
ST = T // P  # 8 t tiles per batch
CAP = 384  # gathered slot capacity per (expert, batch)
SC = CAP // P  # 3 slot tiles
BIG = 1 << 20
F32 = mybir.dt.float32
F32R = mybir.dt.float32r
I32 = mybir.dt.int32
AF = mybir.ActivationFunctionType
OP = mybir.AluOpType
AX = mybir.AxisListType
RED = bass_isa.ReduceOp
STAGE = int(os.environ.get("V3STAGE", "99"))
SUB = int(os.environ.get("V3SUB", "99"))

_CACHE = {}


def _emit(nc, tc, dt_in, dt_out):
    (xT, xn, wg_d, wq_d, wk_d, wv_d, wo_d, bq_d, bk_d, bv_d, bo_d) = dt_in
    (out_d,) = dt_out

    with tc.tile_pool(name="const", bufs=1) as const, tc.tile_pool(
        name="weights", bufs=1
    ) as wpool, tc.tile_pool(name="drams", bufs=1, space="DRAM") as dramp:
        # ---------------- constants ----------------
        wg_sb = const.tile([P, DC, E], F32)
        nc.sync.dma_start(wg_sb[:], wg_d.ap().rearrange("(c p) e -> p c e", p=P))
        bq_sb = const.tile([P, FT], F32)
        nc.sync.dma_start(bq_sb[:], bq_d.ap())
        bk_sb = const.tile([P, FT], F32)
        nc.sync.dma_start(bk_sb[:], bk_d.ap())
        bv_sb = const.tile([P, FT], F32)
        nc.sync.dma_start(bv_sb[:], bv_d.ap())
        bo_sb = const.tile([P, FT], F32)
        nc.sync.dma_start(bo_sb[:], bo_d.ap())

        wos = const.tile([P, FT], F32)  # wo row sums, f chunked
        u_f = const.tile([P, DC], F32)  # u = wv @ wo_sum, d chunked
        uB = const.tile([P, DH], F32)  # u broadcast along partitions
        c0 = const.tile([P, 1], F32)  # bv . wo_sum
        boS = const.tile([P, 1], F32)  # sum(bo)

        # index-machinery constants
        idn = const.tile([P, P], F32)
        make_identity(nc, idn[:])
        iota_f_i = const.tile([P, P], I32)
        nc.gpsimd.iota(iota_f_i[:], pattern=[[1, P]], base=0, channel_multiplier=0)
        iota_p_i = const.tile([P, 1], I32)
        nc.gpsimd.iota(iota_p_i[:], pattern=[[0, 1]], base=0, channel_multiplier=1)
        iota_ff = const.tile([P, P], F32)
        nc.vector.tensor_copy(iota_ff[:], iota_f_i[:])
        iota_pf = const.tile([P, 1], F32)
        nc.vector.tensor_copy(iota_pf[:], iota_p_i[:])
        ltri = const.tile([P, P], F32)  # ltri[k, m] = (m > k)
        nc.vector.tensor_scalar(ltri[:], iota_ff[:], iota_pf[:], None, op0=OP.is_gt)
        iocF_i = const.tile([P, CAP], I32)  # value = free slot index j
        nc.gpsimd.iota(iocF_i[:], pattern=[[1, CAP]], base=0, channel_multiplier=0)
        iocF = const.tile([P, CAP], F32)
        nc.vector.tensor_copy(iocF[:], iocF_i[:])
        iosc_i = const.tile([P, SC], I32)  # value = slot j = c*128 + p
        nc.gpsimd.iota(iosc_i[:], pattern=[[P, SC]], base=0, channel_multiplier=1)
        iosc = const.tile([P, SC], F32)
        nc.vector.tensor_copy(iosc[:], iosc_i[:])
        tv8 = const.tile([P, ST], I32)  # within-batch token id t = c*128 + p
        nc.gpsimd.iota(tv8[:], pattern=[[P, ST]], base=0, channel_multiplier=1)
        bigt = const.tile([P, SC], I32)
        nc.vector.memset(bigt[:], BIG)
        zt = const.tile([P, ST], F32)
        nc.vector.memset(zt[:], 0.0)

        wq_r = wpool.tile([P, DC, DH], F32R)
        wk_r = wpool.tile([P, DC, DH], F32R)

        wosF_d = dramp.tile([DH], F32, tag="wosF", name="wosF")
        uF_d = dramp.tile([DH], F32, tag="uF", name="uF")
        sc_d = dramp.tile([N], F32, tag="scd", name="scd")
        idx_d = [
            dramp.tile([CAP], I32, tag=f"idxd{b}", name=f"idxd{b}")
            for b in range(B)
        ]

        def prep_a(prep):
            for fc in range(FT):
                wc = prep.tile([P, D], F32, tag="rot", name=f"wo{fc}")
                nc.sync.dma_start(wc[:], wo_d.ap()[ts(fc, P), :])
                nc.vector.reduce_sum(wos[:, fc : fc + 1], wc[:], axis=AX.X)
            t1 = prep.tile([P, 1], F32, tag="t1", name="t1")
            nc.vector.reduce_sum(t1[:], bo_sb[:], axis=AX.X)
            nc.gpsimd.partition_all_reduce(
                boS[:], t1[:], channels=P, reduce_op=RED.add
            )
            # zero the token-space scatter target once
            for bb in range(B):
                nc.sync.dma_start(
                    sc_d[bb * T : (bb + 1) * T].rearrange("(c p) -> p c", p=P),
                    zt[:],
                )

        def prep_b(prep):
            t8 = prep.tile([P, FT], F32, tag="t8", name="t8")
            nc.vector.tensor_mul(t8[:], bv_sb[:], wos[:])
            t1b = prep.tile([P, 1], F32, tag="t1", name="t1b")
            nc.vector.reduce_sum(t1b[:], t8[:], axis=AX.X)
            nc.gpsimd.partition_all_reduce(
                c0[:], t1b[:], channels=P, reduce_op=RED.add
            )
            nc.sync.dma_start(wosF_d.rearrange("(c p) -> p c", p=P), wos[:])
            woB = prep.tile([P, DH], F32, tag="wob", name="woB")
            nc.sync.dma_start(woB[:], wosF_d[None, :].to_broadcast([P, DH]))
            for dc in range(DC):
                vc = prep.tile([P, DH], F32, tag="rot", name=f"wv{dc}")
                nc.sync.dma_start(vc[:], wv_d.ap()[ts(dc, P), :])
                nc.vector.tensor_mul(vc[:], vc[:], woB[:])
                nc.vector.reduce_sum(u_f[:, dc : dc + 1], vc[:], axis=AX.X)
            nc.sync.dma_start(uF_d.rearrange("(c p) -> p c", p=P), u_f[:])
            nc.sync.dma_start(uB[:], uF_d[None, :].to_broadcast([P, DH]))

        def prep_w():
            nc.sync.dma_start(
                wq_r[:], wq_d.ap().rearrange("(c p) f -> p c f", p=P)
            )
            nc.sync.dma_start(
                wk_r[:], wk_d.ap().rearrange("(c p) f -> p c f", p=P)
            )

        with tc.tile_pool(name="pb", bufs=1) as pbp, tc.tile_pool(
            name="gx", bufs=1
        ) as gx, tc.tile_pool(name="gsb", bufs=3) as gsb, tc.tile_pool(
            name="prep", bufs=2
        ) as prep, tc.tile_pool(name="xgp", bufs=4) as xgp, tc.tile_pool(
            name="xgt", bufs=1
        ) as xgtp, tc.tile_pool(name="ktq", bufs=1) as ktqp, tc.tile_pool(
            name="qtg", bufs=1
        ) as qtgp, tc.tile_pool(name="eg", bufs=2) as egp, tc.tile_pool(
            name="sm", bufs=2
        ) as sm, tc.tile_pool(name="psA", bufs=1, space="PSUM") as psA, tc.tile_pool(
            name="psB", bufs=1, space="PSUM"
        ) as psB:
            # per-batch persistent tiles
            maskb = [
                pbp.tile([P, ST], F32, tag=f"maskb{b}", name=f"maskb{b}")
                for b in range(B)
            ]
            cwb = [
                pbp.tile([P, ST], F32, tag=f"cwb{b}", name=f"cwb{b}")
                for b in range(B)
            ]
            idxt = [
                pbp.tile([P, SC], I32, tag=f"idxt{b}", name=f"idxt{b}")
                for b in range(B)
            ]
            wv_w = [
                pbp.tile([P, SC], F32, tag=f"wvw{b}", name=f"wvw{b}")
                for b in range(B)
            ]
            omc = [
                pbp.tile([P, SC], F32, tag=f"omc{b}", name=f"omc{b}")
                for b in range(B)
            ]
            omF = [
                pbp.tile([P, CAP], F32, tag=f"omF{b}", name=f"omF{b}")
                for b in range(B)
            ]

            def gate(b):
                xb = []
                for dc in range(DC):
                    xc = gx.tile(
                        [P, T], F32, tag=f"xb{dc}", name=f"xb{b}_{dc}", bufs=1
                    )
                    nc.sync.dma_start(
                        xc[:], xT.ap()[ts(dc, P), b * T : (b + 1) * T]
                    )
                    xb.append(xc)
                for tt in range(ST):
                    pst = psB.tile([P, E], F32, tag="g", bufs=2, name=f"g{b}_{tt}")
                    for dc in range(DC):
                        nc.tensor.matmul(
                            pst[:],
                            xb[dc][:, ts(tt, P)],
                            wg_sb[:, dc],
                            start=(dc == 0),
                            stop=(dc == DC - 1),
                        )
                    gl = gsb.tile([P, E], F32, tag="gl")
                    nc.scalar.activation(gl[:], pst[:], AF.Copy)
                    mx8 = gsb.tile([P, E], F32, tag="mx8")
                    nc.vector.max(out=mx8[:], in_=gl[:])
                    mxn = gsb.tile([P, 1], F32, tag="mxn")
                    nc.vector.tensor_scalar_mul(mxn[:], mx8[:, 0:1], -1.0)
                    probs = gsb.tile([P, E], F32, tag="probs")
                    se = gsb.tile([P, 1], F32, tag="se")
                    nc.scalar.activation(
                        probs[:], gl[:], AF.Exp, bias=mxn[:], scale=1.0,
                        accum_out=se[:],
                    )
                    rs = gsb.tile([P, 1], F32, tag="rs")
                    nc.vector.reciprocal(rs[:], se[:])
                    nc.vector.tensor_scalar(
                        maskb[b][:, tt : tt + 1], gl[:, 0:1], mx8[:, 1:2], None,
                        op0=OP.is_ge,
                    )
                    nc.vector.scalar_tensor_tensor(
                        cwb[b][:, tt : tt + 1],
                        probs[:, 0:1],
                        rs[:],
                        maskb[b][:, tt : tt + 1],
                        op0=OP.mult,
                        op1=OP.mult,
                    )

            def gather_part1(b, prep=None):
                """Index build, x gather, vw, omega (no PE except prefix)."""
                tot = gsb.tile([P, ST], F32, tag="tot")
                nc.gpsimd.partition_all_reduce(
                    tot[:], maskb[b][:], channels=P, reduce_op=RED.add
                )
                carry = gsb.tile([P, ST], F32, tag="carry")
                nc.vector.memset(carry[:, 0:1], 0.0)
                for tt in range(1, ST):
                    nc.vector.tensor_tensor(
                        carry[:, tt : tt + 1],
                        carry[:, tt - 1 : tt],
                        tot[:, tt - 1 : tt],
                        op=OP.add,
                    )
                cf = gsb.tile([P, 1], F32, tag="cf")  # total count C
                nc.vector.tensor_tensor(
                    cf[:], carry[:, ST - 1 : ST], tot[:, ST - 1 : ST], op=OP.add
                )
                if SUB < 2:
                    return []
                # token ids for this batch
                tvb = gsb.tile([P, ST], I32, tag="tvb")
                nc.vector.tensor_scalar(tvb[:], tv8[:], b * T, None, op0=OP.add)
                # prefill index list with BIG
                nc.sync.dma_start(
                    idx_d[b].rearrange("(c p) -> p c", p=P), bigt[:]
                )
                if SUB < 3:
                    return []
                gposi = gsb.tile([P, ST], I32, tag="gposi")
                for tt in range(ST):
                    pp = psB.tile(
                        [P, 1], F32, tag="p1", bufs=1, name=f"pp{b}_{tt}"
                    )
                    nc.tensor.matmul(
                        pp[:],
                        ltri[:],
                        maskb[b][:, tt : tt + 1],
                        start=True,
                        stop=True,
                    )
                    gp = gsb.tile([P, 1], F32, tag="gp")
                    nc.vector.tensor_tensor(
                        gp[:], pp[:], carry[:, tt : tt + 1], op=OP.add
                    )
                    gm = gsb.tile([P, 1], F32, tag="gm")
                    nc.vector.tensor_scalar(
                        gm[:],
                        maskb[b][:, tt : tt + 1],
                        float(-BIG),
                        float(BIG),
                        op0=OP.mult,
                        op1=OP.add,
                    )
                    nc.vector.tensor_add(gm[:], gm[:], gp[:])
                    nc.vector.tensor_copy(gposi[:, tt : tt + 1], gm[:])
                if SUB < 4:
                    return []
                for tt in range(ST):
                    nc.gpsimd.indirect_dma_start(
                        out=idx_d[b][:, None],
                        out_offset=bass.IndirectOffsetOnAxis(
                            ap=gposi[:, tt : tt + 1], axis=0
                        ),
                        in_=tvb[:, tt : tt + 1],
                        in_offset=None,
                        bounds_check=CAP - 1,
                        oob_is_err=False,
                    )
                nc.sync.dma_start(
                    idxt[b][:], idx_d[b].rearrange("(c p) -> p c", p=P)
                )
                if SUB < 5:
                    return []
                # gather x rows; pads remain zero
                xg = []
                for i in range(SC):
                    xgi = xgp.tile([P, D], F32, tag="xg", name=f"xg{b}_{i}")
                    nc.vector.memset(xgi[:], 0.0)
                    nc.gpsimd.indirect_dma_start(
                        out=xgi[:],
                        out_offset=None,
                        in_=xn.ap(),
                        in_offset=bass.IndirectOffsetOnAxis(
                            ap=idxt[b][:, i : i + 1], axis=0
                        ),
                        bounds_check=N - 1,
                        oob_is_err=False,
                    )
                    xg.append(xgi)
                if SUB < 6:
                    return xg
                if SUB < 7:
                    return xg
                # omega: 1 for j < C, (T - C) at j == CAP-1, else 0
                tmc = gsb.tile([P, 1], F32, tag="tmc")
                nc.vector.tensor_scalar(
                    tmc[:], cf[:], -1.0, float(T), op0=OP.mult, op1=OP.add
                )
                rep = gsb.tile([P, SC], F32, tag="rep")
                nc.vector.tensor_scalar(
                    omc[b][:], iosc[:], cf[:], None, op0=OP.is_lt
                )
                nc.vector.tensor_scalar(
                    rep[:], iosc[:], float(CAP - 1), None, op0=OP.is_equal
                )
                nc.vector.tensor_scalar(rep[:], rep[:], tmc[:], None, op0=OP.mult)
                nc.vector.tensor_add(omc[b][:], omc[b][:], rep[:])
                repF = gsb.tile([P, CAP], F32, tag="repF")
                nc.vector.tensor_scalar(
                    omF[b][:], iocF[:], cf[:], None, op0=OP.is_lt
                )
                nc.vector.tensor_scalar(
                    repF[:], iocF[:], float(CAP - 1), None, op0=OP.is_equal
                )
                nc.vector.tensor_scalar(repF[:], repF[:], tmc[:], None, op0=OP.mult)
                nc.vector.tensor_add(omF[b][:], omF[b][:], repF[:])
                return xg

            def vw_calc(b, xg, prep):
                # vw[j] = xg_j . u + c0 (pads -> c0); emitted after prep_b
                # so the uB read follows its write in program order
                vwg = gsb.tile([P, SC], F32, tag="vwg")
                for i in range(SC):
                    scr = prep.tile([P, D], F32, tag="rot", name=f"scr{b}_{i}")
                    nc.vector.tensor_mul(scr[:], xg[i][:], uB[:])
                    nc.vector.reduce_sum(vwg[:, i : i + 1], scr[:], axis=AX.X)
                nc.vector.tensor_scalar_add(vwg[:], vwg[:], c0[:])
                nc.vector.tensor_mul(wv_w[b][:], vwg[:], omc[b][:])

            def transposes(b, xg):
                xgT = xgtp.tile([P, DC, CAP], F32R, tag="xgT", name=f"xgT{b}")
                for i in range(SC):
                    for dc in range(DC):
                        tp = psA.tile(
                            [P, P], F32, tag="tp", bufs=2, name=f"tp{b}_{i}_{dc}"
                        )
                        nc.tensor.transpose(tp[:], xg[i][:, ts(dc, P)], idn[:])
                        nc.scalar.activation(xgT[:, dc, ts(i, P)], tp[:], AF.Copy)
                return xgT

            def proj(b, xgT):
                kTg = ktqp.tile([P, FT, CAP], F32R, tag="kTg", name=f"kTg{b}")
                qTg = qtgp.tile([P, FT, CAP], F32R, tag="qTg", name=f"qTg{b}")
                for dst, w_r, b_sb in ((kTg, wk_r, bk_sb), (qTg, wq_r, bq_sb)):
                    for ft in range(FT):
                        pq = psA.tile(
                            [P, CAP], F32, tag="p384", bufs=3,
                            name=f"pj{b}_{ft}",
                        )
                        for dc in range(DC):
                            nc.tensor.matmul(
                                pq[:],
                                w_r[:, dc, ts(ft, P)],
                                xgT[:, dc],
                                start=(dc == 0),
                                stop=(dc == DC - 1),
                            )
                        nc.scalar.activation(
                            dst[:, ft], pq[:], AF.Identity,
                            bias=b_sb[:, ft : ft + 1],
                        )
                return kTg, qTg

            def attention(b, kTg, qTg):
                Eg = egp.tile([P, SC, CAP], F32, tag="Eg", name=f"Eg{b}")
                erw = gsb.tile([P, SC], F32, tag="erw")
                for st in range(SC):
                    pss = psA.tile(
                        [P, CAP], F32, tag="p384", bufs=3, name=f"sc{b}_{st}"
                    )
                    for dhc in range(FT):
                        nc.tensor.matmul(
                            pss[:],
                            kTg[:, dhc, ts(st, P)],
                            qTg[:, dhc],
                            start=(dhc == 0),
                            stop=(dhc == FT - 1),
                        )
                    nc.scalar.activation(
                        Eg[:, st], pss[:], AF.Exp, scale=float(1.0 / D)
                    )
                    scrE = gsb.tile([P, CAP], F32, tag="scrE")
                    nc.vector.tensor_mul(scrE[:], Eg[:, st], omF[b][:])
                    nc.vector.reduce_sum(erw[:, st : st + 1], scrE[:], axis=AX.X)
                # Z = omega_s . erw
                scr3 = gsb.tile([P, SC], F32, tag="scr3")
                zp = gsb.tile([P, 1], F32, tag="zp")
                nc.vector.tensor_mul(scr3[:], erw[:], omc[b][:])
                nc.vector.reduce_sum(zp[:], scr3[:], axis=AX.X)
                za = gsb.tile([P, 1], F32, tag="za")
                nc.gpsimd.partition_all_reduce(
                    za[:], zp[:], channels=P, reduce_op=RED.add
                )
                rZ = gsb.tile([P, 1], F32, tag="rZ")
                nc.vector.reciprocal(rZ[:], za[:])
                # num[t] = sum_s omega_s E[s, t] vw[s]
                numg = gsb.tile([P, SC], F32, tag="numg")
                for ti in range(SC):
                    pn = psB.tile(
                        [P, 1], F32, tag="p1", bufs=1, name=f"pn{b}_{ti}"
                    )
                    for scc in range(SC):
                        nc.tensor.matmul(
                            pn[:],
                            Eg[:, scc, ts(ti, P)],
                            wv_w[b][:, scc : scc + 1],
                            start=(scc == 0),
                            stop=(scc == SC - 1),
                        )
                    nc.scalar.activation(numg[:, ti : ti + 1], pn[:], AF.Copy)
                # out_sum = num/Z + bo_sum; scatter to token space
                outg = gsb.tile([P, SC], F32, tag="outg")
                nc.vector.tensor_scalar(
                    outg[:], numg[:], rZ[:], boS[:], op0=OP.mult, op1=OP.add
                )
                for i in range(SC):
                    nc.gpsimd.indirect_dma_start(
                        out=sc_d[:, None],
                        out_offset=bass.IndirectOffsetOnAxis(
                            ap=idxt[b][:, i : i + 1], axis=0
                        ),
                        in_=outg[:, i : i + 1],
                        in_offset=None,
                        bounds_check=N - 1,
                        oob_is_err=False,
                    )
                # read back in token-partition layout, weight by cw, emit
                scb = sm.tile([P, ST], F32, tag="scb")
                nc.sync.dma_start(
                    scb[:],
                    sc_d[b * T : (b + 1) * T].rearrange("(c p) -> p c", p=P),
                )
                ob = sm.tile([P, ST], F32, tag="ob")
                nc.vector.tensor_mul(ob[:], scb[:], cwb[b][:])
                nc.sync.dma_start(out_d.ap()[b], ob[:])

            # ---------------- pipeline ----------------
            gate(0)
            gate(1)
            prep_w()
            if STAGE >= 2:
                xg_cur = gather_part1(0, prep)
            prep_a(prep)
            prep_b(prep)
            vw_calc(0, xg_cur, prep)
            for b in range(B):
                if STAGE < 3:
                    break
                xgT = transposes(b, xg_cur)
                if b + 1 < B:
                    if b + 1 >= 2:
                        gate(b + 1)
                    xg_cur = gather_part1(b + 1, prep)
                    vw_calc(b + 1, xg_cur, prep)
                if STAGE >= 4:
                    kTg, qTg = proj(b, xgT)
                if STAGE >= 5:
                    attention(b, kTg, qTg)


def build_nc():
    nc = bacc.Bacc("TRN2", target_bir_lowering=False, debug=False, num_devices=8)
    xT = nc.dram_tensor("xT", [D, N], F32, kind="ExternalInput")
    xn = nc.dram_tensor("xn", [N, D], F32, kind="ExternalInput")
    wg_d = nc.dram_tensor("wg", [D, E], F32, kind="ExternalInput")
    wq_d = nc.dram_tensor("wq", [D, DH], F32R, kind="ExternalInput")
    wk_d = nc.dram_tensor("wk", [D, DH], F32R, kind="ExternalInput")
    wv_d = nc.dram_tensor("wv", [D, DH], F32, kind="ExternalInput")
    wo_d = nc.dram_tensor("wo", [DH, D], F32, kind="ExternalInput")
    bq_d = nc.dram_tensor("bq", [P, FT], F32, kind="ExternalInput")
    bk_d = nc.dram_tensor("bk", [P, FT], F32, kind="ExternalInput")
    bv_d = nc.dram_tensor("bv", [P, FT], F32, kind="ExternalInput")
    bo_d = nc.dram_tensor("bo", [P, FT], F32, kind="ExternalInput")
    out_d = nc.dram_tensor("contrib", [B, P, ST], F32, kind="ExternalOutput")
    with tile.TileContext(nc) as tc:
        _emit(
            nc,
            tc,
            (xT, xn, wg_d, wq_d, wk_d, wv_d, wo_d, bq_d, bk_d, bv_d, bo_d),
            (out_d,),
        )
    nc.compile()
    return nc


def _chunk(v):
    return np.ascontiguousarray(v.reshape(FT, P).T)


def make_in_maps(x, wg, wqkv, bqkv, wo, bo):
    xn = np.ascontiguousarray(x.reshape(N, D))
    xT = np.ascontiguousarray(xn.T)
    in_maps = []
    for e in range(E):
        perm = [e] + [j for j in range(E) if j != e]
        in_maps.append(
            {
                "xT": xT,
                "xn": xn,
                "wg": np.ascontiguousarray(wg[:, perm]),
                "wq": np.ascontiguousarray(wqkv[e][:, 0::3]),
                "wk": np.ascontiguousarray(wqkv[e][:, 1::3]),
                "wv": np.ascontiguousarray(wqkv[e][:, 2::3]),
                "wo": np.ascontiguousarray(wo[e]),
                "bq": _chunk(bqkv[e][0::3]),
                "bk": _chunk(bqkv[e][1::3]),
                "bv": _chunk(bqkv[e][2::3]),
                "bo": _chunk(bo[e]),
            }
        )
    return in_maps


def run_device(in_maps, trace=False):
    if "nc" not in _CACHE:
        _CACHE["nc"] = build_nc()
    return bass_utils.run_bass_kernel_spmd(
        _CACHE["nc"], in_maps, core_ids=list(range(E)), trace=trace
    )


def kernel(x, wg, wqkv, bqkv, wo, bo, top_k):
    assert int(top_k) == 2, f"kernel hardcodes top_k=2, got {top_k}"
    x = np.asarray(x, np.float32)
    wg = np.asarray(wg, np.float32)
    wqkv = np.asarray(wqkv, np.float32)
    bqkv = np.asarray(bqkv, np.float32)
    wo = np.asarray(wo, np.float32)
    bo = np.asarray(bo, np.float32)

    res = run_device(make_in_maps(x, wg, wqkv, bqkv, wo, bo))
    total = np.zeros((B, T), np.float64)
    for c in range(E):
        contrib = res.results[c]["contrib"]  # [B, P, ST], t = tt*128 + p
        total += contrib.transpose(0, 2, 1).reshape(B, T).astype(np.float64)
    m = total.max(axis=1, keepdims=True)
    ls = total - m - np.log(np.exp(total - m).sum(axis=1, keepdims=True))
    return ls.astype(np.float32)



# revision 6
# speedup vs baseline: 1.0382x; 1.0382x over previous
"""MoE-routing attention kernel for 8 Trainium2 NeuronCores (v2).

Expert parallelism (1 expert per core), full inputs in, full output out.
Per core, for its expert e (gate columns host-permuted so col 0 = e):

  gate (fp32 PE, exact): logits = x @ wg per batch, top-2 mask + combine
     weight cw.  fp32 matmul is required: min top2/top3 logit gap on this
     input is 2e-6; fp32r (3.6e-4 hw error) flips decisions.
  gather: exclusive prefix of the mask (one ltri matmul + carry chain)
     -> slot positions; scatter token ids to an idx list in DRAM; gather
     bf16 x rows (pad slots point at a zero row appended to x).
  q/k proj (bf16 PE) on CAP=384 gathered slots; S on the [384,384]
     block; E = exp(S/D) fp32.
  weighting trick: the (T,T)-joint softmax terms for unassigned tokens
     are bias-only; one zero pad slot weighted by (T-C) represents all
     of them.  erw[s] = sum_t om_t E[s,t] comes free from the Exp
     activation's accum_out plus a (T-CAP)*E[s,last] correction.
  v collapse: sum_d out_e[t] = sum_s P[t,s]*vw[s] + sum(bo), with
     vw = x_g . u + c0, u = wv @ rowsum(wo) folded on host (weight-only
     preprocessing), vw computed as one PE matmul row.
  combine: scatter out_sum to token space; one final readback for all
     batches, multiply by cw, emit [P, B*ST].

Host: sums the 8 per-core [B,T] contributions, applies log_softmax.
"""

import os
import sys

import numpy as np

for _p in ("/opt/trn_rl_repo", "/root/.axon_site/_ro/trn_rl_repo"):
    if _p not in sys.path:
        sys.path.append(_p)

import ml_dtypes  # noqa: E402

import concourse.bass as bass  # noqa: E402
import concourse.mybir as mybir  # noqa: E402
import concourse.bass_isa as bass_isa  # noqa: E402
import concourse.tile as tile  # noqa: E402
from concourse import bacc  # noqa: E402
from concourse import bass_utils  # noqa: E402
from concourse.bass import ts  # noqa: E402
from concourse.masks import make_identity  # noqa: E402

P = 128
B, T, D, E = 4, 1024, 1024, 8
DH = D
N = B * T
DC = D // P  # 8 contraction chunks
FT = DH // P  # 8 feature tiles
ST = T // P  # 8 token tiles per batch
CAP = 384  # gathered slot capacity per (expert, batch); max actual 278
SC = CAP // P  # 3 slot tiles
BT = B * ST  # 32 token-tile columns overall
BIG = 1 << 20
F32 = mybir.dt.float32
BF16 = mybir.dt.bfloat16
I32 = mybir.dt.int32
AF = mybir.ActivationFunctionType
OP = mybir.AluOpType
AX = mybir.AxisListType
RED = bass_isa.ReduceOp

_CACHE = {}


def _emit(nc, tc, dt_in, dt_out):
    (xT, xb16_d, wg_d, wq_d, wk_d, u_d, bq_d, bk_d, cb_d,
     ltri_d, iosc_d, tv8_d, nv_d) = dt_in
    (out_d,) = dt_out

    with tc.tile_pool(name="const", bufs=1) as const, tc.tile_pool(
        name="weights", bufs=1
    ) as wpool, tc.tile_pool(name="drams", bufs=1, space="DRAM") as dramp:
        # ---------------- small constants (scalar ring) ----------------
        wg_sb = const.tile([P, DC, E], F32)
        nc.scalar.dma_start(wg_sb[:], wg_d.ap().rearrange("(c p) e -> p c e", p=P))
        bq_sb = const.tile([P, FT], F32)
        nc.scalar.dma_start(bq_sb[:], bq_d.ap())
        bk_sb = const.tile([P, FT], F32)
        nc.scalar.dma_start(bk_sb[:], bk_d.ap())
        cb_sb = const.tile([P, 2], F32)  # col0 c0, col1 boS
        nc.scalar.dma_start(cb_sb[:], cb_d.ap())
        u_sb = const.tile([P, DC], BF16)
        nc.scalar.dma_start(u_sb[:], u_d.ap().rearrange("(c p) x -> p (c x)", p=P))
        ltri = const.tile([P, P], F32)  # ltri[k, m] = (m > k)
        nc.scalar.dma_start(ltri[:], ltri_d.ap())
        iosc = const.tile([P, SC], F32)  # slot id j = c*128 + p
        nc.scalar.dma_start(iosc[:], iosc_d.ap())
        tv8 = const.tile([P, ST], I32)  # within-batch token id
        nc.scalar.dma_start(tv8[:], tv8_d.ap())
        nv = const.tile([P, SC], I32)  # idx prefill value N
        nc.scalar.dma_start(nv[:], nv_d.ap())

        idnb = const.tile([P, P], BF16)
        make_identity(nc, idnb[:])
        ones1 = const.tile([1, 1], F32)
        nc.vector.memset(ones1[:], 1.0)
        repm = const.tile([P, SC], F32)  # indicator(j == CAP-1)
        nc.vector.tensor_scalar(repm[:], iosc[:], float(CAP - 1), None,
                                op0=OP.is_equal)
        zt = const.tile([P, BT], F32)
        nc.vector.memset(zt[:], 0.0)

        # ---------------- big weights (sync ring, FIFO) ----------------
        wq_sb = wpool.tile([P, DC, DH], BF16)
        wk_sb = wpool.tile([P, DC, DH], BF16)

        sc_d = dramp.tile([N], F32, tag="scd", name="scd")
        idx_d = [
            dramp.tile([CAP], I32, tag=f"idxd{b}", name=f"idxd{b}")
            for b in range(B)
        ]

        with tc.tile_pool(name="pb", bufs=1) as pbp, tc.tile_pool(
            name="gx", bufs=2
        ) as gx, tc.tile_pool(name="gsb", bufs=3) as gsb, tc.tile_pool(
            name="xgp", bufs=2
        ) as xgp, tc.tile_pool(name="xgt", bufs=2) as xgtp, tc.tile_pool(
            name="ktq", bufs=2
        ) as ktqp, tc.tile_pool(name="eg", bufs=2) as egp, tc.tile_pool(
            name="ps", bufs=1, space="PSUM"
        ) as ps:
            # persistent per-batch tiles
            maskb = [
                pbp.tile([P, ST], F32, tag=f"maskb{b}", name=f"maskb{b}")
                for b in range(B)
            ]
            idxt = [
                pbp.tile([P, SC], I32, tag=f"idxt{b}", name=f"idxt{b}")
                for b in range(B)
            ]
            omc = [
                pbp.tile([P, SC], F32, tag=f"omc{b}", name=f"omc{b}")
                for b in range(B)
            ]
            cw_all = pbp.tile([P, B, ST], F32, tag="cwall", name="cwall")

            def load_xb(b):
                xb = gx.tile([P, DC, T], F32, tag="xb", name=f"xb{b}")
                nc.sync.dma_start(
                    xb[:],
                    xT.ap()[:, b * T:(b + 1) * T].rearrange(
                        "(c p) t -> p c t", p=P),
                )
                return xb

            def gate(b, xb):
                gl = gsb.tile([P, ST, E], F32, tag="gl")
                mx = gsb.tile([P, ST, 8], F32, tag="mx")
                for tt in range(ST):
                    pst = ps.tile([P, E], F32, tag="g", bufs=2,
                                  name=f"g{b}_{tt}")
                    for dc in range(DC):
                        nc.tensor.matmul(
                            pst[:],
                            xb[:, dc, ts(tt, P)],
                            wg_sb[:, dc],
                            start=(dc == 0),
                            stop=(dc == DC - 1),
                        )
                    nc.scalar.activation(gl[:, tt], pst[:], AF.Copy)
                    nc.vector.max(out=mx[:, tt], in_=gl[:, tt])
                # mask: own logit >= 2nd max (before shifting)
                nc.vector.tensor_tensor(
                    maskb[b][:], gl[:, :, 0], mx[:, :, 1], op=OP.is_ge
                )
                for tt in range(ST):
                    nc.vector.tensor_scalar(
                        gl[:, tt], gl[:, tt], mx[:, tt, 0:1], None,
                        op0=OP.subtract,
                    )
                nc.scalar.activation(gl[:], gl[:], AF.Exp)
                se = gsb.tile([P, ST, 1], F32, tag="se")
                nc.vector.reduce_sum(se[:], gl[:], axis=AX.X)
                rs = gsb.tile([P, ST], F32, tag="rs")
                nc.vector.reciprocal(rs[:], se[:, :, 0])
                nc.vector.tensor_tensor(
                    cw_all[:, b], gl[:, :, 0], rs[:], op=OP.mult
                )
                nc.vector.tensor_mul(cw_all[:, b], cw_all[:, b], maskb[b][:])

            def gather(b):
                tot = gsb.tile([P, ST], F32, tag="tot")
                nc.gpsimd.partition_all_reduce(
                    tot[:], maskb[b][:], channels=P, reduce_op=RED.add
                )
                carry = gsb.tile([P, ST], F32, tag="carry")
                nc.vector.memset(carry[:, 0:1], 0.0)
                for tt in range(1, ST):
                    nc.vector.tensor_tensor(
                        carry[:, tt:tt + 1],
                        carry[:, tt - 1:tt],
                        tot[:, tt - 1:tt],
                        op=OP.add,
                    )
                cf = gsb.tile([P, 1], F32, tag="cf")  # count C
                nc.vector.tensor_tensor(
                    cf[:], carry[:, ST - 1:ST], tot[:, ST - 1:ST], op=OP.add
                )
                # omega weights on slots
                tmc = gsb.tile([P, 1], F32, tag="tmc")  # T - C
                nc.vector.tensor_scalar(
                    tmc[:], cf[:], -1.0, float(T), op0=OP.mult, op1=OP.add
                )
                nc.vector.tensor_scalar(
                    omc[b][:], iosc[:], cf[:], None, op0=OP.is_lt
                )
                nc.vector.scalar_tensor_tensor(
                    omc[b][:], repm[:], tmc[:], omc[b][:],
                    op0=OP.mult, op1=OP.add,
                )
                # token ids, slot positions
                tvb = gsb.tile([P, ST], I32, tag="tvb")
                nc.vector.tensor_scalar(tvb[:], tv8[:], b * T, None,
                                        op0=OP.add)
                pp8 = ps.tile([P, ST], F32, tag="p1", bufs=2,
                              name=f"pp8{b}")
                nc.tensor.matmul(pp8[:], ltri[:], maskb[b][:],
                                 start=True, stop=True)
                gm8 = gsb.tile([P, ST], F32, tag="gm8")
                nc.vector.tensor_scalar(
                    gm8[:], maskb[b][:], float(-BIG), float(BIG),
                    op0=OP.mult, op1=OP.add,
                )
                nc.vector.tensor_add(gm8[:], gm8[:], pp8[:])
                nc.vector.tensor_add(gm8[:], gm8[:], carry[:])
                gposi = gsb.tile([P, ST], I32, tag="gposi")
                nc.vector.tensor_copy(gposi[:], gm8[:])
                # prefill idx with N, scatter token ids to slots
                nc.scalar.dma_start(
                    idx_d[b].rearrange("(c p) -> p c", p=P), nv[:]
                )
                for tt in range(ST):
                    nc.gpsimd.indirect_dma_start(
                        out=idx_d[b][:, None],
                        out_offset=bass.IndirectOffsetOnAxis(
                            ap=gposi[:, tt:tt + 1], axis=0
                        ),
                        in_=tvb[:, tt:tt + 1],
                        in_offset=None,
                        bounds_check=CAP - 1,
                        oob_is_err=False,
                    )
                nc.scalar.dma_start(
                    idxt[b][:], idx_d[b].rearrange("(c p) -> p c", p=P)
                )
                # gather bf16 x rows; pad slots (idx == N) read the zero row
                xg = xgp.tile([P, SC, D], BF16, tag="xg", name=f"xg{b}")
                for i in range(SC):
                    nc.gpsimd.indirect_dma_start(
                        out=xg[:, i],
                        out_offset=None,
                        in_=xb16_d.ap(),
                        in_offset=bass.IndirectOffsetOnAxis(
                            ap=idxt[b][:, i:i + 1], axis=0
                        ),
                        bounds_check=N,
                        oob_is_err=False,
                    )
                return xg

            def transposes(b, xg):
                xgT = xgtp.tile([P, DC, CAP], BF16, tag="xgT", name=f"xgT{b}")
                for i in range(SC):
                    for dc in range(DC):
                        tp = ps.tile([P, P], BF16, tag="tp", bufs=2,
                                     name=f"tp{b}_{i}_{dc}")
                        nc.tensor.transpose(tp[:], xg[:, i, ts(dc, P)],
                                            idnb[:])
                        nc.vector.tensor_copy(xgT[:, dc, ts(i, P)], tp[:])
                return xgT

            def proj(b, xgT):
                kq = ktqp.tile([P, 2, FT, CAP], BF16, tag="kq",
                               name=f"kq{b}")
                for di, (w_sb, b_sb) in enumerate(
                    ((wk_sb, bk_sb), (wq_sb, bq_sb))
                ):
                    for ft in range(FT):
                        pq = ps.tile([P, CAP], F32, tag="p384", bufs=2,
                                     name=f"pj{b}_{di}_{ft}")
                        for dc in range(DC):
                            nc.tensor.matmul(
                                pq[:],
                                w_sb[:, dc, ts(ft, P)],
                                xgT[:, dc],
                                start=(dc == 0),
                                stop=(dc == DC - 1),
                            )
                        nc.scalar.activation(
                            kq[:, di, ft], pq[:], AF.Identity,
                            bias=b_sb[:, ft:ft + 1],
                        )
                return kq

            def vw_calc(b, xgT):
                pvw = ps.tile([1, CAP], F32, tag="p1", bufs=2,
                              name=f"pvw{b}")
                for dc in range(DC):
                    nc.tensor.matmul(
                        pvw[:],
                        u_sb[:, dc:dc + 1],
                        xgT[:, dc],
                        start=(dc == 0),
                        stop=(dc == DC - 1),
                    )
                vws = gsb.tile([1, CAP], F32, tag="vws")
                nc.scalar.activation(vws[:], pvw[:], AF.Identity,
                                     bias=cb_sb[0:1, 0:1])
                wv_w = gsb.tile([P, SC], F32, tag="wvw")
                for i in range(SC):
                    tvp = ps.tile([P, 1], F32, tag="p1", bufs=2,
                                  name=f"tvp{b}_{i}")
                    nc.tensor.transpose(tvp[:], vws[:, ts(i, P)], ones1[:])
                    nc.vector.tensor_tensor(
                        wv_w[:, i:i + 1], tvp[:], omc[b][:, i:i + 1],
                        op=OP.mult,
                    )
                return wv_w

            def attention(b, kq, wv_w):
                Eg = egp.tile([P, SC, CAP], F32, tag="Eg", name=f"Eg{b}")
                erw = gsb.tile([P, SC], F32, tag="erw")
                for st in range(SC):
                    pss = ps.tile([P, CAP], F32, tag="p384", bufs=2,
                                  name=f"sc{b}_{st}")
                    for fc in range(FT):
                        nc.tensor.matmul(
                            pss[:],
                            kq[:, 0, fc, ts(st, P)],
                            kq[:, 1, fc],
                            start=(fc == 0),
                            stop=(fc == FT - 1),
                        )
                    nc.scalar.activation(
                        Eg[:, st], pss[:], AF.Exp, scale=float(1.0 / D),
                        accum_out=erw[:, st:st + 1],
                    )
                # erw[s] = accum + (T - CAP) * E[s, last]
                nc.vector.scalar_tensor_tensor(
                    erw[:], Eg[:, :, CAP - 1], float(T - CAP), erw[:],
                    op0=OP.mult, op1=OP.add,
                )
                # Z = om_s . erw
                scr3 = gsb.tile([P, SC], F32, tag="scr3")
                zp = gsb.tile([P, 1], F32, tag="zp")
                nc.vector.tensor_mul(scr3[:], erw[:], omc[b][:])
                nc.vector.reduce_sum(zp[:], scr3[:], axis=AX.X)
                za = gsb.tile([P, 1], F32, tag="za")
                nc.gpsimd.partition_all_reduce(
                    za[:], zp[:], channels=P, reduce_op=RED.add
                )
                rZ = gsb.tile([P, 1], F32, tag="rZ")
                nc.vector.reciprocal(rZ[:], za[:])
                # num[t] = sum_s om_s E[s,t] vw[s]
                numg = gsb.tile([P, SC], F32, tag="numg")
                for ti in range(SC):
                    pn = ps.tile([P, 1], F32, tag="p1", bufs=2,
                                 name=f"pn{b}_{ti}")
                    for scc in range(SC):
                        nc.tensor.matmul(
                            pn[:],
                            Eg[:, scc, ts(ti, P)],
                            wv_w[:, scc:scc + 1],
                            start=(scc == 0),
                            stop=(scc == SC - 1),
                        )
                    nc.scalar.activation(numg[:, ti:ti + 1], pn[:], AF.Copy)
                outg = gsb.tile([P, SC], F32, tag="outg")
                nc.vector.tensor_scalar(
                    outg[:], numg[:], rZ[:], cb_sb[:, 1:2],
                    op0=OP.mult, op1=OP.add,
                )
                # scatter to token space; pad slots (idx == N) are skipped
                for i in range(SC):
                    nc.gpsimd.indirect_dma_start(
                        out=sc_d[:, None],
                        out_offset=bass.IndirectOffsetOnAxis(
                            ap=idxt[b][:, i:i + 1], axis=0
                        ),
                        in_=outg[:, i:i + 1],
                        in_offset=None,
                        bounds_check=N - 1,
                        oob_is_err=False,
                    )

            # ---------------- pipeline ----------------
            nc.scalar.dma_start(
                sc_d.rearrange("(x p) -> p x", p=P), zt[:]
            )
            xb_cur = load_xb(0)
            nc.sync.dma_start(
                wq_sb[:], wq_d.ap().rearrange("(c p) f -> p c f", p=P)
            )
            nc.sync.dma_start(
                wk_sb[:], wk_d.ap().rearrange("(c p) f -> p c f", p=P)
            )
            xb_nxt = load_xb(1)
            gate(0, xb_cur)
            xg_cur = gather(0)
            xgT_cur = transposes(0, xg_cur)
            for b in range(B):
                kq = proj(b, xgT_cur)
                wv_w = vw_calc(b, xgT_cur)
                if b + 1 < B:
                    gate(b + 1, xb_nxt)
                    if b + 2 < B:
                        xb_nxt = load_xb(b + 2)
                    xg_nxt = gather(b + 1)
                attention(b, kq, wv_w)
                if b + 1 < B:
                    xgT_cur = transposes(b + 1, xg_nxt)

            # final combine: readback all batches, weight by cw, emit
            scv = gsb.tile([P, BT], F32, tag="scv")
            nc.scalar.dma_start(scv[:], sc_d.rearrange("(x p) -> p x", p=P))
            ob = gsb.tile([P, BT], F32, tag="ob")
            nc.vector.tensor_mul(ob[:], scv[:], cw_all[:])
            nc.scalar.dma_start(out_d.ap(), ob[:])


def build_nc():
    nc = bacc.Bacc("TRN2", target_bir_lowering=False, debug=False,
                   num_devices=8)
    xT = nc.dram_tensor("xT", [D, N], F32, kind="ExternalInput")
    xb16_d = nc.dram_tensor("xb16", [N + 1, D], BF16, kind="ExternalInput")
    wg_d = nc.dram_tensor("wg", [D, E], F32, kind="ExternalInput")
    wq_d = nc.dram_tensor("wq", [D, DH], BF16, kind="ExternalInput")
    wk_d = nc.dram_tensor("wk", [D, DH], BF16, kind="ExternalInput")
    u_d = nc.dram_tensor("u", [D, 1], BF16, kind="ExternalInput")
    bq_d = nc.dram_tensor("bq", [P, FT], F32, kind="ExternalInput")
    bk_d = nc.dram_tensor("bk", [P, FT], F32, kind="ExternalInput")
    cb_d = nc.dram_tensor("cb", [P, 2], F32, kind="ExternalInput")
    ltri_d = nc.dram_tensor("ltri", [P, P], F32, kind="ExternalInput")
    iosc_d = nc.dram_tensor("iosc", [P, SC], F32, kind="ExternalInput")
    tv8_d = nc.dram_tensor("tv8", [P, ST], I32, kind="ExternalInput")
    nv_d = nc.dram_tensor("nv", [P, SC], I32, kind="ExternalInput")
    out_d = nc.dram_tensor("contrib", [P, BT], F32, kind="ExternalOutput")
    with tile.TileContext(nc) as tc:
        _emit(
            nc,
            tc,
            (xT, xb16_d, wg_d, wq_d, wk_d, u_d, bq_d, bk_d, cb_d,
             ltri_d, iosc_d, tv8_d, nv_d),
            (out_d,),
        )
    nc.compile()
    return nc


def _chunk(v):
    return np.ascontiguousarray(v.reshape(FT, P).T.astype(np.float32))


def make_in_maps(x, wg, wqkv, bqkv, wo, bo):
    xn = np.ascontiguousarray(x.reshape(N, D), dtype=np.float32)
    xT = np.ascontiguousarray(xn.T)
    xb16 = np.zeros((N + 1, D), dtype=ml_dtypes.bfloat16)
    xb16[:N] = xn.astype(ml_dtypes.bfloat16)

    iop = np.arange(P, dtype=np.int64)
    iosc = (iop[:, None] + 128 * np.arange(SC)[None, :]).astype(np.float32)
    tv8 = (iop[:, None] + 128 * np.arange(ST)[None, :]).astype(np.int32)
    nv = np.full((P, SC), N, dtype=np.int32)
    ltri = (iop[None, :] > iop[:, None]).astype(np.float32)  # [k, m] = m > k

    in_maps = []
    for e in range(E):
        perm = [e] + [j for j in range(E) if j != e]
        wq = wqkv[e][:, 0::3].astype(np.float32)
        wk = wqkv[e][:, 1::3].astype(np.float32)
        wv = wqkv[e][:, 2::3].astype(np.float64)
        bq = bqkv[e][0::3].astype(np.float32)
        bk = bqkv[e][1::3].astype(np.float32)
        bv = bqkv[e][2::3].astype(np.float64)
        wos = wo[e].astype(np.float64).sum(axis=1)
        u = (wv @ wos).astype(ml_dtypes.bfloat16).reshape(D, 1)
        c0 = float(bv @ wos)
        boS = float(bo[e].astype(np.float64).sum())
        cb = np.zeros((P, 2), dtype=np.float32)
        cb[:, 0] = c0
        cb[:, 1] = boS
        in_maps.append(
            {
                "xT": xT,
                "xb16": xb16,
                "wg": np.ascontiguousarray(wg[:, perm], dtype=np.float32),
                "wq": np.ascontiguousarray(wq.astype(ml_dtypes.bfloat16)),
                "wk": np.ascontiguousarray(wk.astype(ml_dtypes.bfloat16)),
                "u": u,
                "bq": _chunk(bq),
                "bk": _chunk(bk),
                "cb": cb,
                "ltri": ltri,
                "iosc": iosc,
                "tv8": tv8,
                "nv": nv,
            }
        )
    return in_maps


def run_device(in_maps, trace=False):
    if "nc" not in _CACHE:
        _CACHE["nc"] = build_nc()
    return bass_utils.run_bass_kernel_spmd(
        _CACHE["nc"], in_maps, core_ids=list(range(E)), trace=trace
    )


def kernel(x, wg, wqkv, bqkv, wo, bo, top_k):
    assert int(top_k) == 2, f"kernel hardcodes top_k=2, got {top_k}"
    x = np.asarray(x, np.float32)
    wg = np.asarray(wg, np.float32)
    wqkv = np.asarray(wqkv, np.float32)
    bqkv = np.asarray(bqkv, np.float32)
    wo = np.asarray(wo, np.float32)
    bo = np.asarray(bo, np.float32)

    res = run_device(make_in_maps(x, wg, wqkv, bqkv, wo, bo))
    total = np.zeros((B, T), np.float64)
    for c in range(E):
        contrib = res.results[c]["contrib"]  # [P, B*ST], col = b*ST + tt
        z = contrib.reshape(P, B, ST).transpose(1, 2, 0).reshape(B, T)
        total += z.astype(np.float64)
    m = total.max(axis=1, keepdims=True)
    ls = total - m - np.log(np.exp(total - m).sum(axis=1, keepdims=True))
    return ls.astype(np.float32)


# revision 12
# speedup vs baseline: 1.4388x; 1.3859x over previous
"""MoE-routing attention kernel for 8 Trainium2 NeuronCores (v2).

Expert parallelism (1 expert per core), full inputs in, full output out.
Per core, for its expert e (gate columns host-permuted so col 0 = e):

  gate (fp32 PE, exact): logits = x @ wg per batch, top-2 mask + combine
     weight cw.  fp32 matmul is required: min top2/top3 logit gap on this
     input is 2e-6; fp32r (3.6e-4 hw error) flips decisions.
  gather: exclusive prefix of the mask (one ltri matmul + carry chain)
     -> slot positions; scatter token ids to an idx list in DRAM; gather
     bf16 x rows (pad slots point at a zero row appended to x).
  q/k proj (bf16 PE) on CAP=384 gathered slots; S on the [384,384]
     block; E = exp(S/D) fp32.
  weighting trick: the (T,T)-joint softmax terms for unassigned tokens
     are bias-only; one zero pad slot weighted by (T-C) represents all
     of them.  erw[s] = sum_t om_t E[s,t] comes free from the Exp
     activation's accum_out plus a (T-CAP)*E[s,last] correction.
  v collapse: sum_d out_e[t] = sum_s P[t,s]*vw[s] + sum(bo), with
     vw = x_g . u + c0, u = wv @ rowsum(wo) folded on host (weight-only
     preprocessing), vw computed as one PE matmul row.
  combine: scatter out_sum to token space; one final readback for all
     batches, multiply by cw, emit [P, B*ST].

Host: sums the 8 per-core [B,T] contributions, applies log_softmax.
"""

import os
import sys

import numpy as np

for _p in ("/opt/trn_rl_repo", "/root/.axon_site/_ro/trn_rl_repo"):
    if _p not in sys.path:
        sys.path.append(_p)

import ml_dtypes  # noqa: E402

import concourse.bass as bass  # noqa: E402
import concourse.mybir as mybir  # noqa: E402
import concourse.bass_isa as bass_isa  # noqa: E402
import concourse.tile as tile  # noqa: E402
from concourse import bacc  # noqa: E402
from concourse import bass_utils  # noqa: E402
from concourse.bass import ts  # noqa: E402
from concourse.masks import make_identity  # noqa: E402

P = 128
B, T, D, E = 4, 1024, 1024, 8
DH = D
N = B * T
DC = D // P  # 8 contraction chunks
FT = DH // P  # 8 feature tiles
ST = T // P  # 8 token tiles per batch
CAP = 384  # gathered slot capacity per (expert, batch); max actual 278
SC = CAP // P  # 3 slot tiles
BT = B * ST  # 32 token-tile columns overall
BIG = 1 << 20
F32 = mybir.dt.float32
BF16 = mybir.dt.bfloat16
I32 = mybir.dt.int32
AF = mybir.ActivationFunctionType
OP = mybir.AluOpType
AX = mybir.AxisListType
RED = bass_isa.ReduceOp

_CACHE = {}


def _emit(nc, tc, dt_in, dt_out):
    (xT, xb16_d, wg_d, wq_d, wk_d, u_d, bq_d, bk_d, cb_d,
     ltri_d, iosc_d, tv8_d, nv_d) = dt_in
    (out_d, dbg_mask, dbg_cw, dbg_idx, dbg_omc, dbg_scv) = dt_out

    with tc.tile_pool(name="const", bufs=1) as const, tc.tile_pool(
        name="weights", bufs=1
    ) as wpool, tc.tile_pool(name="drams", bufs=1, space="DRAM") as dramp:
        # ---------------- small constants (scalar ring) ----------------
        wg_sb = const.tile([P, DC, E], F32)
        nc.scalar.dma_start(wg_sb[:], wg_d.ap())
        bq_sb = const.tile([P, FT], F32)
        nc.scalar.dma_start(bq_sb[:], bq_d.ap())
        bk_sb = const.tile([P, FT], F32)
        nc.scalar.dma_start(bk_sb[:], bk_d.ap())
        cb_sb = const.tile([P, 2], F32)  # col0 c0, col1 boS
        nc.scalar.dma_start(cb_sb[:], cb_d.ap())
        u_sb = const.tile([P, DC], BF16)
        nc.scalar.dma_start(u_sb[:], u_d.ap())
        ltri = const.tile([P, P], F32)  # ltri[k, m] = (m > k)
        nc.scalar.dma_start(ltri[:], ltri_d.ap())
        iosc = const.tile([P, SC], F32)  # slot id j = c*128 + p
        nc.scalar.dma_start(iosc[:], iosc_d.ap())
        tv8 = const.tile([P, ST], I32)  # within-batch token id
        nc.scalar.dma_start(tv8[:], tv8_d.ap())
        nv = const.tile([P, SC], I32)  # idx prefill value N
        nc.scalar.dma_start(nv[:], nv_d.ap())

        idnb = const.tile([P, P], BF16)
        make_identity(nc, idnb[:])
        ones1 = const.tile([1, 1], F32)
        nc.vector.memset(ones1[:], 1.0)
        repm = const.tile([P, SC], F32)  # indicator(j == CAP-1)
        nc.vector.tensor_scalar(repm[:], iosc[:], float(CAP - 1), None,
                                op0=OP.is_equal)
        zt = const.tile([P, BT], F32)
        nc.vector.memset(zt[:], 0.0)

        # ---------------- big weights (sync ring, FIFO) ----------------
        wq_sb = wpool.tile([P, DC, DH], BF16)
        wk_sb = wpool.tile([P, DC, DH], BF16)

        sc_d = dramp.tile([N], F32, tag="scd", name="scd")
        idx_d = [
            dramp.tile([CAP], I32, tag=f"idxd{b}", name=f"idxd{b}")
            for b in range(B)
        ]

        with tc.tile_pool(name="pb", bufs=1) as pbp, tc.tile_pool(
            name="gx", bufs=2
        ) as gx, tc.tile_pool(name="gsb", bufs=3) as gsb, tc.tile_pool(
            name="xgp", bufs=2
        ) as xgp, tc.tile_pool(name="xgt", bufs=2) as xgtp, tc.tile_pool(
            name="ktq", bufs=2
        ) as ktqp, tc.tile_pool(name="eg", bufs=2) as egp, tc.tile_pool(
            name="ps", bufs=1, space="PSUM"
        ) as ps:
            # persistent per-batch tiles
            maskb = [
                pbp.tile([P, ST], F32, tag=f"maskb{b}", name=f"maskb{b}")
                for b in range(B)
            ]
            idxt = [
                pbp.tile([P, SC], I32, tag=f"idxt{b}", name=f"idxt{b}")
                for b in range(B)
            ]
            omc = [
                pbp.tile([P, SC], F32, tag=f"omc{b}", name=f"omc{b}")
                for b in range(B)
            ]
            idxr = [
                pbp.tile([P, SC], I32, tag=f"idxr{b}", name=f"idxr{b}")
                for b in range(B)
            ]
            cw_all = pbp.tile([P, B, ST], F32, tag="cwall", name="cwall")

            def load_xb(b):
                xb = gx.tile([P, DC, T], F32, tag="xb", name=f"xb{b}")
                nc.sync.dma_start(
                    xb[:],
                    xT.ap()[:, b * T:(b + 1) * T].rearrange(
                        "(c p) t -> p c t", p=P),
                )
                return xb

            def gate(b, xb):
                gl = gsb.tile([P, ST, E], F32, tag="gl")
                mx = gsb.tile([P, ST, 8], F32, tag="mx")
                for tt in range(ST):
                    pst = ps.tile([P, E], F32, tag="g", bufs=2,
                                  name=f"g{b}_{tt}")
                    for dc in range(DC):
                        nc.tensor.matmul(
                            pst[:],
                            xb[:, dc, ts(tt, P)],
                            wg_sb[:, dc],
                            start=(dc == 0),
                            stop=(dc == DC - 1),
                        )
                    nc.scalar.activation(gl[:, tt], pst[:], AF.Copy)
                    nc.vector.max(out=mx[:, tt], in_=gl[:, tt])
                # mask: own logit >= 2nd max (before shifting)
                nc.vector.tensor_tensor(
                    maskb[b][:], gl[:, :, 0], mx[:, :, 1], op=OP.is_ge
                )
                for tt in range(ST):
                    nc.vector.tensor_scalar(
                        gl[:, tt], gl[:, tt], mx[:, tt, 0:1], None,
                        op0=OP.subtract,
                    )
                nc.scalar.activation(gl[:], gl[:], AF.Exp)
                se = gsb.tile([P, ST, 1], F32, tag="se")
                nc.vector.reduce_sum(se[:], gl[:], axis=AX.X)
                rs = gsb.tile([P, ST], F32, tag="rs")
                nc.vector.reciprocal(rs[:], se[:, :, 0])
                nc.vector.tensor_tensor(
                    cw_all[:, b], gl[:, :, 0], rs[:], op=OP.mult
                )
                nc.vector.tensor_mul(cw_all[:, b], cw_all[:, b], maskb[b][:])

            def gather(b):
                tot = gsb.tile([P, ST], F32, tag="tot")
                nc.gpsimd.partition_all_reduce(
                    tot[:], maskb[b][:], channels=P, reduce_op=RED.add
                )
                carry = gsb.tile([P, ST], F32, tag="carry")
                nc.vector.memset(carry[:, 0:1], 0.0)
                for tt in range(1, ST):
                    nc.vector.tensor_tensor(
                        carry[:, tt:tt + 1],
                        carry[:, tt - 1:tt],
                        tot[:, tt - 1:tt],
                        op=OP.add,
                    )
                cf = gsb.tile([P, 1], F32, tag="cf")  # count C
                nc.vector.tensor_tensor(
                    cf[:], carry[:, ST - 1:ST], tot[:, ST - 1:ST], op=OP.add
                )
                # omega weights on slots
                tmc = gsb.tile([P, 1], F32, tag="tmc")  # T - C
                nc.vector.tensor_scalar(
                    tmc[:], cf[:], -1.0, float(T), op0=OP.mult, op1=OP.add
                )
                nc.vector.tensor_scalar(
                    omc[b][:], iosc[:], cf[:], None, op0=OP.is_lt
                )
                nc.vector.scalar_tensor_tensor(
                    omc[b][:], repm[:], tmc[:], omc[b][:],
                    op0=OP.mult, op1=OP.add,
                )
                # token ids, slot positions
                tvb = gsb.tile([P, ST], I32, tag="tvb")
                nc.vector.tensor_scalar(tvb[:], tv8[:], b * T, None,
                                        op0=OP.add)
                pp8 = ps.tile([P, ST], F32, tag="p1", bufs=2,
                              name=f"pp8{b}")
                nc.tensor.matmul(pp8[:], ltri[:], maskb[b][:],
                                 start=True, stop=True)
                gm8 = gsb.tile([P, ST], F32, tag="gm8")
                nc.vector.tensor_scalar(
                    gm8[:], maskb[b][:], float(-BIG), float(BIG),
                    op0=OP.mult, op1=OP.add,
                )
                nc.vector.tensor_add(gm8[:], gm8[:], pp8[:])
                nc.vector.tensor_add(gm8[:], gm8[:], carry[:])
                gposi = gsb.tile([P, ST], I32, tag="gposi")
                nc.vector.tensor_copy(gposi[:], gm8[:])
                # idx_d is partition-major [P, SC] (addr = p*SC + c for slot
                # j = c*128 + p): transform slot j -> jr = (j & 127)*SC + j>>7
                jra = gsb.tile([P, ST], I32, tag="jra")
                nc.vector.tensor_scalar(
                    jra[:], gposi[:], 127, None, op0=OP.bitwise_and
                )
                nc.vector.tensor_scalar(
                    jra[:], jra[:], SC, None, op0=OP.mult
                )
                jrb = gsb.tile([P, ST], I32, tag="jrb")
                nc.vector.tensor_scalar(
                    jrb[:], gposi[:], 7, None, op0=OP.logical_shift_right
                )
                nc.vector.tensor_add(jra[:], jra[:], jrb[:])
                # prefill idx with N, scatter token ids to slots
                nc.scalar.dma_start(idx_d[b].rearrange("(p c) -> p c", p=P),
                                    nv[:])
                for tt in range(ST):
                    nc.gpsimd.indirect_dma_start(
                        out=idx_d[b][:, None],
                        out_offset=bass.IndirectOffsetOnAxis(
                            ap=jra[:, tt:tt + 1], axis=0
                        ),
                        in_=tvb[:, tt:tt + 1],
                        in_offset=None,
                        bounds_check=CAP - 1,
                        oob_is_err=False,
                    )
                nc.scalar.dma_start(
                    idxt[b][:], idx_d[b].rearrange("(p c) -> p c", p=P)
                )
                # scatter-back offsets: token t -> rt = (t & 127)*BT + t>>7,
                # pads (t == N) pushed out of bounds
                ra = gsb.tile([P, SC], I32, tag="ra")
                nc.vector.tensor_scalar(
                    ra[:], idxt[b][:], 127, None, op0=OP.bitwise_and
                )
                nc.vector.tensor_scalar(
                    ra[:], ra[:], BT, None, op0=OP.mult
                )
                rb = gsb.tile([P, SC], I32, tag="rb")
                nc.vector.tensor_scalar(
                    rb[:], idxt[b][:], 7, None, op0=OP.logical_shift_right
                )
                nc.vector.tensor_add(ra[:], ra[:], rb[:])
                sel = gsb.tile([P, SC], I32, tag="sel")
                nc.vector.tensor_scalar(
                    sel[:], idxt[b][:], N - 1, None, op0=OP.is_gt
                )
                nc.vector.scalar_tensor_tensor(
                    idxr[b][:], sel[:], BIG, ra[:], op0=OP.mult, op1=OP.add
                )
                # gather bf16 x rows; pad slots (idx == N) read the zero row
                xg = xgp.tile([P, SC, D], BF16, tag="xg", name=f"xg{b}")
                for i in range(SC):
                    nc.gpsimd.indirect_dma_start(
                        out=xg[:, i],
                        out_offset=None,
                        in_=xb16_d.ap(),
                        in_offset=bass.IndirectOffsetOnAxis(
                            ap=idxt[b][:, i:i + 1], axis=0
                        ),
                        bounds_check=N,
                        oob_is_err=False,
                    )
                return xg

            def transposes(b, xg):
                xgT = xgtp.tile([P, DC, CAP], BF16, tag="xgT", name=f"xgT{b}")
                for i in range(SC):
                    for dc in range(DC):
                        tp = ps.tile([P, P], BF16, tag="tp", bufs=2,
                                     name=f"tp{b}_{i}_{dc}")
                        nc.tensor.transpose(tp[:], xg[:, i, ts(dc, P)],
                                            idnb[:])
                        nc.vector.tensor_copy(xgT[:, dc, ts(i, P)], tp[:])
                return xgT

            def proj(b, xgT):
                kq = ktqp.tile([P, 2, FT, CAP], BF16, tag="kq",
                               name=f"kq{b}")
                for di, (w_sb, b_sb) in enumerate(
                    ((wk_sb, bk_sb), (wq_sb, bq_sb))
                ):
                    for ft in range(FT):
                        pq = ps.tile([P, CAP], F32, tag="p384", bufs=2,
                                     name=f"pj{b}_{di}_{ft}")
                        for dc in range(DC):
                            nc.tensor.matmul(
                                pq[:],
                                w_sb[:, dc, ts(ft, P)],
                                xgT[:, dc],
                                start=(dc == 0),
                                stop=(dc == DC - 1),
                            )
                        nc.scalar.activation(
                            kq[:, di, ft], pq[:], AF.Identity,
                            bias=b_sb[:, ft:ft + 1],
                        )
                return kq

            def vw_calc(b, xgT):
                pvw = ps.tile([1, CAP], F32, tag="p1", bufs=2,
                              name=f"pvw{b}")
                for dc in range(DC):
                    nc.tensor.matmul(
                        pvw[:],
                        u_sb[:, dc:dc + 1],
                        xgT[:, dc],
                        start=(dc == 0),
                        stop=(dc == DC - 1),
                    )
                vws = gsb.tile([1, CAP], F32, tag="vws")
                nc.scalar.activation(vws[:], pvw[:], AF.Identity,
                                     bias=cb_sb[0:1, 0:1])
                wv_w = gsb.tile([P, SC], F32, tag="wvw")
                for i in range(SC):
                    tvp = ps.tile([P, 1], F32, tag="p1", bufs=2,
                                  name=f"tvp{b}_{i}")
                    nc.tensor.transpose(tvp[:], vws[:, ts(i, P)], ones1[:])
                    nc.vector.tensor_tensor(
                        wv_w[:, i:i + 1], tvp[:], omc[b][:, i:i + 1],
                        op=OP.mult,
                    )
                return wv_w

            def attention(b, kq, wv_w):
                Eg = egp.tile([P, SC, CAP], F32, tag="Eg", name=f"Eg{b}")
                erw = gsb.tile([P, SC], F32, tag="erw")
                for st in range(SC):
                    pss = ps.tile([P, CAP], F32, tag="p384", bufs=2,
                                  name=f"sc{b}_{st}")
                    for fc in range(FT):
                        nc.tensor.matmul(
                            pss[:],
                            kq[:, 0, fc, ts(st, P)],
                            kq[:, 1, fc],
                            start=(fc == 0),
                            stop=(fc == FT - 1),
                        )
                    nc.scalar.activation(
                        Eg[:, st], pss[:], AF.Exp, scale=float(1.0 / D),
                        accum_out=erw[:, st:st + 1],
                    )
                # erw[s] = accum + (T - CAP) * E[s, last]
                nc.vector.scalar_tensor_tensor(
                    erw[:], Eg[:, :, CAP - 1], float(T - CAP), erw[:],
                    op0=OP.mult, op1=OP.add,
                )
                # Z = om_s . erw
                scr3 = gsb.tile([P, SC], F32, tag="scr3")
                zp = gsb.tile([P, 1], F32, tag="zp")
                nc.vector.tensor_mul(scr3[:], erw[:], omc[b][:])
                nc.vector.reduce_sum(zp[:], scr3[:], axis=AX.X)
                za = gsb.tile([P, 1], F32, tag="za")
                nc.gpsimd.partition_all_reduce(
                    za[:], zp[:], channels=P, reduce_op=RED.add
                )
                rZ = gsb.tile([P, 1], F32, tag="rZ")
                nc.vector.reciprocal(rZ[:], za[:])
                # num[t] = sum_s om_s E[s,t] vw[s]
                numg = gsb.tile([P, SC], F32, tag="numg")
                for ti in range(SC):
                    pn = ps.tile([P, 1], F32, tag="p1", bufs=2,
                                 name=f"pn{b}_{ti}")
                    for scc in range(SC):
                        nc.tensor.matmul(
                            pn[:],
                            Eg[:, scc, ts(ti, P)],
                            wv_w[:, scc:scc + 1],
                            start=(scc == 0),
                            stop=(scc == SC - 1),
                        )
                    nc.scalar.activation(numg[:, ti:ti + 1], pn[:], AF.Copy)
                outg = gsb.tile([P, SC], F32, tag="outg")
                nc.vector.tensor_scalar(
                    outg[:], numg[:], rZ[:], cb_sb[:, 1:2],
                    op0=OP.mult, op1=OP.add,
                )
                # scatter to token space; pad slots are out of bounds
                for i in range(SC):
                    nc.gpsimd.indirect_dma_start(
                        out=sc_d[:, None],
                        out_offset=bass.IndirectOffsetOnAxis(
                            ap=idxr[b][:, i:i + 1], axis=0
                        ),
                        in_=outg[:, i:i + 1],
                        in_offset=None,
                        bounds_check=N - 1,
                        oob_is_err=False,
                    )

            # ---------------- pipeline ----------------
            nc.scalar.dma_start(sc_d.rearrange("(p x) -> p x", p=P),
                                zt[:])
            xb_cur = load_xb(0)
            nc.sync.dma_start(
                wq_sb[:], wq_d.ap().rearrange("(c p) f -> p c f", p=P)
            )
            nc.sync.dma_start(
                wk_sb[:], wk_d.ap().rearrange("(c p) f -> p c f", p=P)
            )
            xb_nxt = load_xb(1)
            gate(0, xb_cur)
            xg_cur = gather(0)
            xgT_cur = transposes(0, xg_cur)
            for b in range(B):
                if b + 1 < B:
                    gate(b + 1, xb_nxt)
                    if b + 2 < B:
                        xb_nxt = load_xb(b + 2)
                    xg_nxt = gather(b + 1)
                kq = proj(b, xgT_cur)
                wv_w = vw_calc(b, xgT_cur)
                attention(b, kq, wv_w)
                if b + 1 < B:
                    xgT_cur = transposes(b + 1, xg_nxt)

            # debug dumps
            for b in range(B):
                nc.scalar.dma_start(dbg_mask.ap()[:, b], maskb[b][:])
                nc.scalar.dma_start(dbg_idx.ap()[:, b], idxt[b][:])
                nc.scalar.dma_start(dbg_omc.ap()[:, b], omc[b][:])
            nc.scalar.dma_start(dbg_cw.ap(), cw_all[:])
            # final combine: readback all batches, weight by cw, emit
            scv = gsb.tile([P, BT], F32, tag="scv")
            nc.scalar.dma_start(scv[:], sc_d.rearrange("(p x) -> p x", p=P))
            nc.scalar.dma_start(dbg_scv.ap(), scv[:])
            ob = gsb.tile([P, BT], F32, tag="ob")
            nc.vector.tensor_mul(ob[:], scv[:], cw_all[:])
            nc.scalar.dma_start(out_d.ap(), ob[:])


def build_nc():
    nc = bacc.Bacc("TRN2", target_bir_lowering=False, debug=False,
                   num_devices=8)
    xT = nc.dram_tensor("xT", [D, N], F32, kind="ExternalInput")
    xb16_d = nc.dram_tensor("xb16", [N + 1, D], BF16, kind="ExternalInput")
    wg_d = nc.dram_tensor("wg", [D, E], F32, kind="ExternalInput")
    wq_d = nc.dram_tensor("wq", [D, DH], BF16, kind="ExternalInput")
    wk_d = nc.dram_tensor("wk", [D, DH], BF16, kind="ExternalInput")
    u_d = nc.dram_tensor("u", [D, 1], BF16, kind="ExternalInput")
    bq_d = nc.dram_tensor("bq", [P, FT], F32, kind="ExternalInput")
    bk_d = nc.dram_tensor("bk", [P, FT], F32, kind="ExternalInput")
    cb_d = nc.dram_tensor("cb", [P, 2], F32, kind="ExternalInput")
    ltri_d = nc.dram_tensor("ltri", [P, P], F32, kind="ExternalInput")
    iosc_d = nc.dram_tensor("iosc", [P, SC], F32, kind="ExternalInput")
    tv8_d = nc.dram_tensor("tv8", [P, ST], I32, kind="ExternalInput")
    nv_d = nc.dram_tensor("nv", [P, SC], I32, kind="ExternalInput")
    out_d = nc.dram_tensor("contrib", [P, BT], F32, kind="ExternalOutput")
    dbg_mask = nc.dram_tensor("dbg_mask", [P, B, ST], F32, kind="ExternalOutput")
    dbg_cw = nc.dram_tensor("dbg_cw", [P, B, ST], F32, kind="ExternalOutput")
    dbg_idx = nc.dram_tensor("dbg_idx", [P, B, SC], I32, kind="ExternalOutput")
    dbg_omc = nc.dram_tensor("dbg_omc", [P, B, SC], F32, kind="ExternalOutput")
    dbg_scv = nc.dram_tensor("dbg_scv", [P, BT], F32, kind="ExternalOutput")
    with tile.TileContext(nc) as tc:
        _emit(
            nc,
            tc,
            (xT, xb16_d, wg_d, wq_d, wk_d, u_d, bq_d, bk_d, cb_d,
             ltri_d, iosc_d, tv8_d, nv_d),
            (out_d, dbg_mask, dbg_cw, dbg_idx, dbg_omc, dbg_scv),
        )
    nc.compile()
    return nc


def _chunk(v):
    return np.ascontiguousarray(v.reshape(FT, P).T.astype(np.float32))


def make_in_maps(x, wg, wqkv, bqkv, wo, bo):
    xn = np.ascontiguousarray(x.reshape(N, D), dtype=np.float32)
    xT = np.ascontiguousarray(xn.T)
    xb16 = np.zeros((N + 1, D), dtype=ml_dtypes.bfloat16)
    xb16[:N] = xn.astype(ml_dtypes.bfloat16)

    iop = np.arange(P, dtype=np.int64)
    iosc = (iop[:, None] + 128 * np.arange(SC)[None, :]).astype(np.float32)
    tv8 = (iop[:, None] + 128 * np.arange(ST)[None, :]).astype(np.int32)
    nv = np.full((P, SC), N, dtype=np.int32)
    ltri = (iop[None, :] > iop[:, None]).astype(np.float32)  # [k, m] = m > k

    in_maps = []
    for e in range(E):
        perm = [e] + [j for j in range(E) if j != e]
        wq = wqkv[e][:, 0::3].astype(np.float32)
        wk = wqkv[e][:, 1::3].astype(np.float32)
        wv = wqkv[e][:, 2::3].astype(np.float64)
        bq = bqkv[e][0::3].astype(np.float32)
        bk = bqkv[e][1::3].astype(np.float32)
        bv = bqkv[e][2::3].astype(np.float64)
        wos = wo[e].astype(np.float64).sum(axis=1)
        u = np.ascontiguousarray(
            (wv @ wos).astype(ml_dtypes.bfloat16).reshape(DC, P).T
        ).reshape(D, 1)
        c0 = float(bv @ wos)
        boS = float(bo[e].astype(np.float64).sum())
        cb = np.zeros((P, 2), dtype=np.float32)
        cb[:, 0] = c0
        cb[:, 1] = boS
        in_maps.append(
            {
                "xT": xT,
                "xb16": xb16,
                "wg": np.ascontiguousarray(
                    wg[:, perm].astype(np.float32).reshape(DC, P, E)
                    .transpose(1, 0, 2)
                ).reshape(D, E),
                "wq": np.ascontiguousarray(wq.astype(ml_dtypes.bfloat16)),
                "wk": np.ascontiguousarray(wk.astype(ml_dtypes.bfloat16)),
                "u": u,
                "bq": _chunk(bq),
                "bk": _chunk(bk),
                "cb": cb,
                "ltri": ltri,
                "iosc": iosc,
                "tv8": tv8,
                "nv": nv,
            }
        )
    return in_maps


def run_device(in_maps, trace=False):
    if "nc" not in _CACHE:
        _CACHE["nc"] = build_nc()
    return bass_utils.run_bass_kernel_spmd(
        _CACHE["nc"], in_maps, core_ids=list(range(E)), trace=trace
    )


def kernel(x, wg, wqkv, bqkv, wo, bo, top_k):
    assert int(top_k) == 2, f"kernel hardcodes top_k=2, got {top_k}"
    x = np.asarray(x, np.float32)
    wg = np.asarray(wg, np.float32)
    wqkv = np.asarray(wqkv, np.float32)
    bqkv = np.asarray(bqkv, np.float32)
    wo = np.asarray(wo, np.float32)
    bo = np.asarray(bo, np.float32)

    res = run_device(make_in_maps(x, wg, wqkv, bqkv, wo, bo))
    total = np.zeros((B, T), np.float64)
    for c in range(E):
        contrib = res.results[c]["contrib"]  # [P, B*ST], col = b*ST + tt
        z = contrib.reshape(P, B, ST).transpose(1, 2, 0).reshape(B, T)
        total += z.astype(np.float64)
    m = total.max(axis=1, keepdims=True)
    ls = total - m - np.log(np.exp(total - m).sum(axis=1, keepdims=True))
    return ls.astype(np.float32)
